# revision 26
# baseline (speedup 1.0000x reference)
"""Distributed Trainium2 kernel for dual (global + local-window) attention.

Sharding: 8 cores = 4 batches x 2 head-groups (4 heads each).
Per core: compute qkv for its heads, global attention P = softmax(q k^T * s)
([4,2048,2048] written to DRAM), x_global = P v, local 16-window attention,
and a partial output projection. Host sums the two partial projections per
batch and adds the bias.

Layouts on device (all per core):
  qT, kT  [128(4h x 32d), 2048(n)]  - head-dim on partitions
  v_sb    16 tiles [128(m), 128(4h x 32d)]
  Pass A (per head h, n-tile nt): S[n,m] tiles -> exp(accum rowsum) ->
     tensor_scalar by 1/rowsum -> DMA to attn[h, n, m] (natural layout).
  Pass B (per n-chunk of 512): S^T[m,n] tiles -> exp -> AV matmuls
     accumulate x_g^T[128(4h x 32d), 512] + masked diagonal blocks give the
     local-window AV and window sums; normalize, combine -> xc^T.
  Proj: pout[n,e] = xc^T.T @ Wp_rows  (partial; host adds pair + bias).
"""

import sys

for p in ("/opt/trn_rl_repo",):
    if p not in sys.path:
        sys.path.insert(0, p)

from contextlib import ExitStack

import numpy as np

import concourse.bass as bass
import concourse.tile as tile
from concourse import mybir
from concourse.bass_utils import run_bass_kernel_spmd
from concourse.masks import make_identity

F32 = mybir.dt.float32
EXP = mybir.ActivationFunctionType.Exp
MUL = mybir.AluOpType.mult

B, N, C = 4, 2048, 256
H = 8
DH = C // H          # 32
NHC = 4              # heads per core
WIN = 16
SCALE = DH ** -0.5
NT = N // 128        # 16 n-tiles
NC_CH = N // 512     # 4 n-chunks
MC = N // 512        # 4 m-chunks (pass A)


def _split_multiwait(nc):
    """This walrus build rejects >1 sync-wait per instruction; hoist extra
    waits onto single-wait NoOps inserted just before (same engine, same
    program order -> semantics preserved)."""
    n_split = 0
    for fn in nc.m.functions:
        for blk in fn.blocks:
            out = []
            for inst in blk.instructions:
                si = inst.sync_info
                if si is not None and si.on_wait is not None and len(si.on_wait) > 1:
                    waits = list(si.on_wait)
                    for w in waits[:-1]:
                        out.append(mybir.InstNoOp(
                            name=f"{inst.name}-wsplit{n_split}",
                            engine=inst.engine,
                            bass_nofuse=True,
                            sync_info=mybir.SyncInfo(on_wait=[w], on_update=[]),
                        ))
                        n_split += 1
                    inst.sync_info = mybir.SyncInfo(
                        on_wait=[waits[-1]], on_update=list(si.on_update or []))
                out.append(inst)
            blk.instructions = out
    return n_split


# stage toggles for perf bisection (bench only; kernel() uses all-on)
FLAGS = {"passA": True, "attn_dma": True, "passB": True, "proj": True,
         "accum": True, "tsmul_ap": True, "mmonly": False}


def _build():
    nc = bass.Bass()

    xT = nc.declare_dram_parameter("xT", [C, N], F32, isOutput=False)
    wq = nc.declare_dram_parameter("wq", [C, 128], F32, isOutput=False)
    wk = nc.declare_dram_parameter("wk", [C, 128], F32, isOutput=False)
    wv = nc.declare_dram_parameter("wv", [C, 128], F32, isOutput=False)
    wp = nc.declare_dram_parameter("wp", [128, C], F32, isOutput=False)
    m16 = nc.declare_dram_parameter("m16", [128, 128], F32, isOutput=False)
    ones32 = nc.declare_dram_parameter("ones32", [128, 32], F32, isOutput=False)
    msel = nc.declare_dram_parameter("msel", [64, 128], F32, isOutput=False)
    attn = nc.declare_dram_parameter("attn", [NHC, N, N], F32, isOutput=True)
    pout = nc.declare_dram_parameter("pout", [N, C], F32, isOutput=True)

    with ExitStack() as ctx:
        tc = ctx.enter_context(tile.TileContext(nc))
        singles = ctx.enter_context(tc.tile_pool(name="singles", bufs=1))
        ea_pool = ctx.enter_context(tc.tile_pool(name="ea", bufs=24))
        et_pool = ctx.enter_context(tc.tile_pool(name="et", bufs=12))
        eloc_pool = ctx.enter_context(tc.tile_pool(name="eloc", bufs=8))
        acc_pool = ctx.enter_context(tc.tile_pool(name="acc", bufs=8))
        sm_pool = ctx.enter_context(tc.tile_pool(name="sm", bufs=8))
        stg_pool = ctx.enter_context(tc.tile_pool(name="stg", bufs=6))
        ps_s = ctx.enter_context(tc.tile_pool(name="ps_s", bufs=4, space="PSUM"))
        ps_acc = ctx.enter_context(tc.tile_pool(name="ps_acc", bufs=3, space="PSUM"))
        ps_misc = ctx.enter_context(tc.tile_pool(name="ps_misc", bufs=1, space="PSUM"))

        # ---- load constants / inputs ----
        xT_sb = [singles.tile([128, N], F32, tag=f"xT{i}", name=f"xT{i}") for i in range(2)]
        for i in range(2):
            nc.sync.dma_start(xT_sb[i][:], xT[i * 128:(i + 1) * 128, :])
        w_sb = {}
        for name, hnd in (("wq", wq), ("wk", wk), ("wv", wv)):
            w_sb[name] = [singles.tile([128, 128], F32, tag=f"{name}{i}", name=f"{name}{i}") for i in range(2)]
            for i in range(2):
                nc.sync.dma_start(w_sb[name][i][:], hnd[i * 128:(i + 1) * 128, :])
        wp_sb = singles.tile([128, C], F32, tag="wp", name="wp_sb")
        nc.sync.dma_start(wp_sb[:], wp[:, :])
        m16_sb = singles.tile([128, 128], F32, tag="m16", name="m16_sb")
        nc.sync.dma_start(m16_sb[:], m16[:, :])
        ones_sb = singles.tile([128, 32], F32, tag="ones", name="ones_sb")
        nc.sync.dma_start(ones_sb[:], ones32[:, :])
        ident = singles.tile([128, 128], F32, tag="ident", name="ident")
        make_identity(nc, ident[:])
        msel_sb = [singles.tile([16, 128], F32, tag=f"msel{j}", name=f"msel{j}") for j in range(4)]
        for j in range(4):
            nc.sync.dma_start(msel_sb[j][:], msel[j * 16:(j + 1) * 16, :])

        # ---- qkv projection ----
        qT_sb = singles.tile([128, N], F32, tag="qT", name="qT_sb")
        kT_sb = singles.tile([128, N], F32, tag="kT", name="kT_sb")
        for dst, wname in ((qT_sb, "wq"), (kT_sb, "wk")):
            for j in range(4):
                ps = ps_s.tile([128, 512], F32, tag="ps_s", name="ps_s")
                for ck in range(2):
                    nc.tensor.matmul(
                        ps[:], w_sb[wname][ck][:], xT_sb[ck][:, j * 512:(j + 1) * 512],
                        start=(ck == 0), stop=(ck == 1))
                nc.scalar.copy(dst[:, j * 512:(j + 1) * 512], ps[:])
        v_sb = [singles.tile([128, 128], F32, tag=f"v{mt}", name=f"v{mt}") for mt in range(NT)]
        for mt in range(NT):
            ps = ps_s.tile([128, 128], F32, tag="ps_s", name="ps_s")
            for ck in range(2):
                nc.tensor.matmul(
                    ps[:], xT_sb[ck][:, mt * 128:(mt + 1) * 128], w_sb["wv"][ck][:],
                    start=(ck == 0), stop=(ck == 1))
            nc.scalar.copy(v_sb[mt][:], ps[:])

        # persistent small state
        r_all = singles.tile([128, 64], F32, tag="r_all", name="r_all")   # col = 4*nt + h
        xc_sb = singles.tile([128, N], F32, tag="xc", name="xc_sb")       # x_comb^T

        for chunk in range(NC_CH):
            # ---------------- pass A: P (natural [n, m] layout) ----------------
            for nt in range(chunk * 4, chunk * 4 + 4) if FLAGS["passA"] else []:
                ea = {}
                acc = [acc_pool.tile([128, 4], F32, tag="acc", name="acc") for _ in range(NHC)]
                if not FLAGS["accum"]:
                    for h in range(NHC):
                        nc.vector.memset(acc[h][:], 1.0)
                for mc in range(MC):
                    pss = [ps_s.tile([128, 512], F32, tag="ps_s", name="ps_s") for _ in range(NHC)]
                    for h in range(NHC):
                        nc.tensor.matmul(
                            pss[h][:],
                            qT_sb[32 * h:32 * h + 32, nt * 128:(nt + 1) * 128],
                            kT_sb[32 * h:32 * h + 32, mc * 512:(mc + 1) * 512],
                            start=True, stop=True, tile_position=(32 * h, 0))
                    if FLAGS["mmonly"]:
                        continue
                    for h in range(NHC):
                        t = ea_pool.tile([128, 512], F32, tag="ea", name="ea")
                        ea[(h, mc)] = t
                        nc.scalar.activation(
                            t[:], pss[h][:], EXP, scale=SCALE,
                            accum_out=acc[h][:, mc:mc + 1] if FLAGS["accum"] else None)
                if FLAGS["mmonly"]:
                    continue
                for h in range(NHC):
                    ssum = sm_pool.tile([128, 1], F32, tag="ssum", name="ssum")
                    nc.vector.tensor_reduce(
                        ssum[:], acc[h][:], mybir.AxisListType.X, mybir.AluOpType.add)
                    rcol = r_all[:, 4 * nt + h:4 * nt + h + 1]
                    nc.vector.reciprocal(rcol, ssum[:])
                    for mc in range(MC):
                        t = ea[(h, mc)]
                        nc.vector.tensor_scalar_mul(
                            t[:], t[:], rcol if FLAGS["tsmul_ap"] else 1.0)
                        if FLAGS["attn_dma"]:
                            nc.sync.dma_start(
                                attn[h, nt * 128:(nt + 1) * 128, mc * 512:(mc + 1) * 512],
                                t[:])

            # ---------------- pass B: x_global^T + local window ----------------
            if not FLAGS["passB"]:
                continue
            # broadcast of 1/s for this chunk: r_all cols {16h + 4*chunk + j}
            rT_ps = ps_misc.tile([16, 128], F32, tag="ps_misc", name="ps_misc")
            r_view = r_all[:, 16 * chunk:16 * chunk + 16]
            nc.tensor.transpose(rT_ps[:], r_view, ident[:])
            rT_sb = stg_pool.tile([16, 128], F32, tag="rT", name="rT_sb")
            nc.scalar.copy(rT_sb[:], rT_ps[:])
            rg_ps = ps_misc.tile([128, 512], F32, tag="ps_misc", name="ps_misc")
            for j in range(4):
                nc.tensor.matmul(
                    rg_ps[:, j * 128:(j + 1) * 128],
                    msel_sb[j][:], rT_sb[:],
                    start=True, stop=True)
            rg_sb = stg_pool.tile([128, 512], F32, tag="rg", name="rg_sb")
            nc.scalar.copy(rg_sb[:], rg_ps[:])

            xg_ps = ps_acc.tile([128, 512], F32, tag="ps_acc", name="ps_acc")
            xl_ps = ps_acc.tile([128, 512], F32, tag="ps_acc", name="ps_acc")
            sl_ps = ps_acc.tile([128, 512], F32, tag="ps_acc", name="ps_acc")
            for mt in range(NT):
                pss = [ps_s.tile([128, 512], F32, tag="ps_s", name="ps_s") for _ in range(NHC)]
                for h in range(NHC):
                    nc.tensor.matmul(
                        pss[h][:],
                        kT_sb[32 * h:32 * h + 32, mt * 128:(mt + 1) * 128],
                        qT_sb[32 * h:32 * h + 32, chunk * 512:(chunk + 1) * 512],
                        start=True, stop=True, tile_position=(32 * h, 0))
                ets = []
                for h in range(NHC):
                    t = et_pool.tile([128, 512], F32, tag="et", name="et")
                    ets.append(t)
                    nc.scalar.activation(t[:], pss[h][:], EXP, scale=SCALE)
                for h in range(NHC):
                    nc.tensor.matmul(
                        xg_ps[32 * h:32 * h + 32, :],
                        v_sb[mt][:, 32 * h:32 * h + 32], ets[h][:],
                        start=(mt == 0), stop=(mt == NT - 1),
                        tile_position=(0, 32 * h))
                if mt // 4 == chunk:
                    j = mt - 4 * chunk
                    for h in range(NHC):
                        el = eloc_pool.tile([128, 128], F32, tag="eloc", name="eloc")
                        nc.vector.tensor_tensor(
                            el[:], ets[h][:, j * 128:(j + 1) * 128], m16_sb[:], MUL)
                        nc.tensor.matmul(
                            xl_ps[32 * h:32 * h + 32, j * 128:(j + 1) * 128],
                            v_sb[mt][:, 32 * h:32 * h + 32], el[:],
                            start=True, stop=True, tile_position=(0, 32 * h))
                        nc.tensor.matmul(
                            sl_ps[32 * h:32 * h + 32, j * 128:(j + 1) * 128],
                            ones_sb[:, :], el[:],
                            start=True, stop=True, tile_position=(0, 32 * h))

            xc_slice = xc_sb[:, chunk * 512:(chunk + 1) * 512]
            nc.vector.tensor_mul(xc_slice, xg_ps[:], rg_sb[:])
            rl_sb = stg_pool.tile([128, 512], F32, tag="rl", name="rl_sb")
            nc.vector.reciprocal(rl_sb[:], sl_ps[:])
            tmp = stg_pool.tile([128, 512], F32, tag="tmp", name="tmp")
            nc.vector.tensor_mul(tmp[:], xl_ps[:], rl_sb[:])
            nc.vector.tensor_add(xc_slice, xc_slice, tmp[:])

        # ---------------- output projection (partial) ----------------
        if not FLAGS["passB"]:
            nc.gpsimd.memset(xc_sb[:], 0.0)
        for nt in range(NT) if FLAGS["proj"] else []:
            pp = ps_s.tile([128, C], F32, tag="ps_s", name="ps_s")
            nc.tensor.matmul(
                pp[:], xc_sb[:, nt * 128:(nt + 1) * 128], wp_sb[:],
                start=True, stop=True)
            ot = stg_pool.tile([128, C], F32, tag="ot", name="ot")
            nc.vector.tensor_copy(ot[:], pp[:])
            nc.sync.dma_start(pout[nt * 128:(nt + 1) * 128, :], ot[:])

    _split_multiwait(nc)
    return nc


_NC_CACHE = None


def _get_nc():
    global _NC_CACHE
    if _NC_CACHE is None:
        _NC_CACHE = _build()
    return _NC_CACHE


def _make_in_maps(inputs):
    x = np.ascontiguousarray(np.asarray(inputs["x"], dtype=np.float32))
    W_qkv = np.asarray(inputs["W_qkv"], dtype=np.float32)
    W_proj = np.asarray(inputs["W_proj"], dtype=np.float32)

    Wq, Wk, Wv = W_qkv[:, :C], W_qkv[:, C:2 * C], W_qkv[:, 2 * C:]
    m16 = np.zeros((128, 128), dtype=np.float32)
    for w in range(128 // WIN):
        m16[w * WIN:(w + 1) * WIN, w * WIN:(w + 1) * WIN] = 1.0
    ones32 = np.ones((128, 32), dtype=np.float32)
    # Rg broadcast selector: rg[32h+d, 128j+p] = rT[4j+h, p]
    msel = np.zeros((64, 128), dtype=np.float32)
    for j in range(4):
        for hh in range(4):
            msel[16 * j + 4 * j + hh, 32 * hh:32 * (hh + 1)] = 1.0

    in_maps = []
    for c in range(8):
        b, hg = c // 2, c % 2
        cols = slice(hg * 128, (hg + 1) * 128)
        in_maps.append({
            "xT": np.ascontiguousarray(x[b].T),
            "wq": np.ascontiguousarray(Wq[:, cols]),
            "wk": np.ascontiguousarray(Wk[:, cols]),
            "wv": np.ascontiguousarray(Wv[:, cols]),
            "wp": np.ascontiguousarray(W_proj[cols, :]),
            "m16": m16,
            "ones32": ones32,
            "msel": msel,
        })
    return in_maps


def kernel(x, W_qkv, W_proj, b_proj):
    b_proj = np.asarray(b_proj, dtype=np.float32)
    in_maps = _make_in_maps({"x": x, "W_qkv": W_qkv, "W_proj": W_proj})

    nc = _get_nc()
    res = run_bass_kernel_spmd(nc, in_maps, core_ids=list(range(8)))
    if res.exec_time_ns is not None:
        print(f"HW exec time: {res.exec_time_ns} ns")

    weights = np.empty((B, H, N, N), dtype=np.float32)
    x_out = np.empty((B, N, C), dtype=np.float32)
    for c in range(8):
        b, hg = c // 2, c % 2
        weights[b, hg * NHC:(hg + 1) * NHC] = res.results[c]["attn"]
    for b in range(B):
        x_out[b] = res.results[2 * b]["pout"] + res.results[2 * b + 1]["pout"] + b_proj
    return (x_out, weights)


# revision 29
# speedup vs baseline: 1.3806x; 1.3806x over previous
"""Distributed Trainium2 kernel for dual (global + local-window) attention.

Sharding: 8 cores = 4 batches x 2 head-groups (4 heads each).
Per core: compute qkv for its heads, global attention P = softmax(q k^T * s)
([4,2048,2048] written to DRAM), x_global = P v, local 16-window attention,
and a partial output projection. Host sums the two partial projections per
batch and adds the bias.

Layouts on device (all per core):
  qT, kT  [128(4h x 32d), 2048(n)]  - head-dim on partitions
  v_sb    16 tiles [128(m), 128(4h x 32d)]
  Pass A (per head h, n-tile nt): S[n,m] tiles -> exp(accum rowsum) ->
     tensor_scalar by 1/rowsum -> DMA to attn[h, n, m] (natural layout).
  Pass B (per n-chunk of 512): S^T[m,n] tiles -> exp -> AV matmuls
     accumulate x_g^T[128(4h x 32d), 512] + masked diagonal blocks give the
     local-window AV and window sums; normalize, combine -> xc^T.
  Proj: pout[n,e] = xc^T.T @ Wp_rows  (partial; host adds pair + bias).
"""

import sys

for p in ("/opt/trn_rl_repo",):
    if p not in sys.path:
        sys.path.insert(0, p)

from contextlib import ExitStack

import numpy as np

import concourse.bass as bass
import concourse.tile as tile
from concourse import mybir
from concourse.bass_utils import run_bass_kernel_spmd
from concourse.masks import make_identity

F32 = mybir.dt.float32
EXP = mybir.ActivationFunctionType.Exp
MUL = mybir.AluOpType.mult

B, N, C = 4, 2048, 256
H = 8
DH = C // H          # 32
NHC = 4              # heads per core
WIN = 16
SCALE = DH ** -0.5
NT = N // 128        # 16 n-tiles
NC_CH = N // 512     # 4 n-chunks
MC = N // 512        # 4 m-chunks (pass A)


def _split_multiwait(nc):
    """This walrus build rejects >1 sync-wait per instruction; hoist extra
    waits onto single-wait NoOps inserted just before (same engine, same
    program order -> semantics preserved)."""
    n_split = 0
    for fn in nc.m.functions:
        for blk in fn.blocks:
            out = []
            for inst in blk.instructions:
                si = inst.sync_info
                if si is not None and si.on_wait is not None and len(si.on_wait) > 1:
                    waits = list(si.on_wait)
                    for w in waits[:-1]:
                        out.append(mybir.InstNoOp(
                            name=f"{inst.name}-wsplit{n_split}",
                            engine=inst.engine,
                            bass_nofuse=True,
                            sync_info=mybir.SyncInfo(on_wait=[w], on_update=[]),
                        ))
                        n_split += 1
                    inst.sync_info = mybir.SyncInfo(
                        on_wait=[waits[-1]], on_update=list(si.on_update or []))
                out.append(inst)
            blk.instructions = out
    return n_split


# stage toggles for perf bisection (bench only; kernel() uses all-on)
FLAGS = {"passA": True, "attn_dma": True, "passB": True, "proj": True,
         "accum": True, "tsmul_ap": True, "mmonly": False, "body_reps": 1}


def _build():
    nc = bass.Bass()

    xT = nc.declare_dram_parameter("xT", [C, N], F32, isOutput=False)
    wq = nc.declare_dram_parameter("wq", [C, 128], F32, isOutput=False)
    wk = nc.declare_dram_parameter("wk", [C, 128], F32, isOutput=False)
    wv = nc.declare_dram_parameter("wv", [C, 128], F32, isOutput=False)
    wp = nc.declare_dram_parameter("wp", [128, C], F32, isOutput=False)
    m16 = nc.declare_dram_parameter("m16", [128, 128], F32, isOutput=False)
    ones32 = nc.declare_dram_parameter("ones32", [128, 32], F32, isOutput=False)
    msel = nc.declare_dram_parameter("msel", [64, 128], F32, isOutput=False)
    attn = nc.declare_dram_parameter("attn", [NHC, N, N], F32, isOutput=True)
    pout = nc.declare_dram_parameter("pout", [N, C], F32, isOutput=True)

    with ExitStack() as ctx:
        tc = ctx.enter_context(tile.TileContext(nc))
        singles = ctx.enter_context(tc.tile_pool(name="singles", bufs=1))
        ea_pool = ctx.enter_context(tc.tile_pool(name="ea", bufs=24))
        et_pool = ctx.enter_context(tc.tile_pool(name="et", bufs=12))
        eloc_pool = ctx.enter_context(tc.tile_pool(name="eloc", bufs=8))
        acc_pool = ctx.enter_context(tc.tile_pool(name="acc", bufs=8))
        sm_pool = ctx.enter_context(tc.tile_pool(name="sm", bufs=8))
        stg_pool = ctx.enter_context(tc.tile_pool(name="stg", bufs=6))
        ps_s = ctx.enter_context(tc.tile_pool(name="ps_s", bufs=4, space="PSUM"))
        ps_acc = ctx.enter_context(tc.tile_pool(name="ps_acc", bufs=3, space="PSUM"))
        ps_misc = ctx.enter_context(tc.tile_pool(name="ps_misc", bufs=1, space="PSUM"))

        # ---- load constants / inputs ----
        xT_sb = [singles.tile([128, N], F32, tag=f"xT{i}", name=f"xT{i}") for i in range(2)]
        for i in range(2):
            nc.sync.dma_start(xT_sb[i][:], xT[i * 128:(i + 1) * 128, :])
        w_sb = {}
        for name, hnd in (("wq", wq), ("wk", wk), ("wv", wv)):
            w_sb[name] = [singles.tile([128, 128], F32, tag=f"{name}{i}", name=f"{name}{i}") for i in range(2)]
            for i in range(2):
                nc.sync.dma_start(w_sb[name][i][:], hnd[i * 128:(i + 1) * 128, :])
        wp_sb = singles.tile([128, C], F32, tag="wp", name="wp_sb")
        nc.sync.dma_start(wp_sb[:], wp[:, :])
        m16_sb = singles.tile([128, 128], F32, tag="m16", name="m16_sb")
        nc.sync.dma_start(m16_sb[:], m16[:, :])
        ones_sb = singles.tile([128, 32], F32, tag="ones", name="ones_sb")
        nc.sync.dma_start(ones_sb[:], ones32[:, :])
        ident = singles.tile([128, 128], F32, tag="ident", name="ident")
        make_identity(nc, ident[:])
        msel_sb = [singles.tile([16, 128], F32, tag=f"msel{j}", name=f"msel{j}") for j in range(4)]
        for j in range(4):
            nc.sync.dma_start(msel_sb[j][:], msel[j * 16:(j + 1) * 16, :])

        for _rep in range(FLAGS["body_reps"]):
            _body(nc, tc, singles, ea_pool, et_pool, eloc_pool, acc_pool,
                  sm_pool, stg_pool, ps_s, ps_acc, ps_misc,
                  xT_sb, w_sb, wp_sb, m16_sb, ones_sb, ident, msel_sb,
                  attn, pout)

    _split_multiwait(nc)
    return nc


def _body(nc, tc, singles, ea_pool, et_pool, eloc_pool, acc_pool,
          sm_pool, stg_pool, ps_s, ps_acc, ps_misc,
          xT_sb, w_sb, wp_sb, m16_sb, ones_sb, ident, msel_sb,
          attn, pout):
    # ---- qkv projection ----
    qT_sb = singles.tile([128, N], F32, tag="qT", name="qT_sb")
    kT_sb = singles.tile([128, N], F32, tag="kT", name="kT_sb")
    for dst, wname in ((qT_sb, "wq"), (kT_sb, "wk")):
        for j in range(4):
            ps = ps_s.tile([128, 512], F32, tag="ps_s", name="ps_s")
            for ck in range(2):
                nc.tensor.matmul(
                    ps[:], w_sb[wname][ck][:], xT_sb[ck][:, j * 512:(j + 1) * 512],
                    start=(ck == 0), stop=(ck == 1))
            nc.scalar.copy(dst[:, j * 512:(j + 1) * 512], ps[:])
    v_sb = [singles.tile([128, 128], F32, tag=f"v{mt}", name=f"v{mt}") for mt in range(NT)]
    for mt in range(NT):
        ps = ps_s.tile([128, 128], F32, tag="ps_s", name="ps_s")
        for ck in range(2):
            nc.tensor.matmul(
                ps[:], xT_sb[ck][:, mt * 128:(mt + 1) * 128], w_sb["wv"][ck][:],
                start=(ck == 0), stop=(ck == 1))
        nc.scalar.copy(v_sb[mt][:], ps[:])

    # persistent small state
    r_all = singles.tile([128, 64], F32, tag="r_all", name="r_all")   # col = 4*nt + h
    xc_sb = singles.tile([128, N], F32, tag="xc", name="xc_sb")       # x_comb^T

    for chunk in range(NC_CH):
        # ---------------- pass A: P (natural [n, m] layout) ----------------
        for nt in range(chunk * 4, chunk * 4 + 4) if FLAGS["passA"] else []:
            ea = {}
            acc = [acc_pool.tile([128, 4], F32, tag="acc", name="acc") for _ in range(NHC)]
            if not FLAGS["accum"]:
                for h in range(NHC):
                    nc.vector.memset(acc[h][:], 1.0)
            for mc in range(MC):
                pss = [ps_s.tile([128, 512], F32, tag="ps_s", name="ps_s") for _ in range(NHC)]
                for h in range(NHC):
                    nc.tensor.matmul(
                        pss[h][:],
                        qT_sb[32 * h:32 * h + 32, nt * 128:(nt + 1) * 128],
                        kT_sb[32 * h:32 * h + 32, mc * 512:(mc + 1) * 512],
                        start=True, stop=True, tile_position=(32 * h, 0))
                if FLAGS["mmonly"]:
                    continue
                for h in range(NHC):
                    t = ea_pool.tile([128, 512], F32, tag="ea", name="ea")
                    ea[(h, mc)] = t
                    nc.scalar.activation(
                        t[:], pss[h][:], EXP, scale=SCALE,
                        accum_out=acc[h][:, mc:mc + 1] if FLAGS["accum"] else None)
            if FLAGS["mmonly"]:
                continue
            for h in range(NHC):
                ssum = sm_pool.tile([128, 1], F32, tag="ssum", name="ssum")
                nc.vector.tensor_reduce(
                    ssum[:], acc[h][:], mybir.AxisListType.X, mybir.AluOpType.add)
                rcol = r_all[:, 4 * nt + h:4 * nt + h + 1]
                nc.vector.reciprocal(rcol, ssum[:])
                for mc in range(MC):
                    t = ea[(h, mc)]
                    nc.vector.tensor_scalar_mul(
                        t[:], t[:], rcol if FLAGS["tsmul_ap"] else 1.0)
                    if FLAGS["attn_dma"]:
                        nc.sync.dma_start(
                            attn[h, nt * 128:(nt + 1) * 128, mc * 512:(mc + 1) * 512],
                            t[:])

        # ---------------- pass B: x_global^T + local window ----------------
        if not FLAGS["passB"]:
            continue
        # broadcast of 1/s for this chunk: r_all cols 16*chunk..16*chunk+16
        rT_ps = ps_misc.tile([16, 128], F32, tag="ps_misc", name="ps_misc")
        r_view = r_all[:, 16 * chunk:16 * chunk + 16]
        nc.tensor.transpose(rT_ps[:], r_view, ident[:])
        rT_sb = stg_pool.tile([16, 128], F32, tag="rT", name="rT_sb")
        nc.scalar.copy(rT_sb[:], rT_ps[:])
        rg_ps = ps_misc.tile([128, 512], F32, tag="ps_misc", name="ps_misc")
        for j in range(4):
            nc.tensor.matmul(
                rg_ps[:, j * 128:(j + 1) * 128],
                msel_sb[j][:], rT_sb[:],
                start=True, stop=True)
        rg_sb = stg_pool.tile([128, 512], F32, tag="rg", name="rg_sb")
        nc.scalar.copy(rg_sb[:], rg_ps[:])

        xg_ps = ps_acc.tile([128, 512], F32, tag="ps_acc", name="ps_acc")
        xl_ps = ps_acc.tile([128, 512], F32, tag="ps_acc", name="ps_acc")
        sl_ps = ps_acc.tile([128, 512], F32, tag="ps_acc", name="ps_acc")
        for mt in range(NT):
            pss = [ps_s.tile([128, 512], F32, tag="ps_s", name="ps_s") for _ in range(NHC)]
            for h in range(NHC):
                nc.tensor.matmul(
                    pss[h][:],
                    kT_sb[32 * h:32 * h + 32, mt * 128:(mt + 1) * 128],
                    qT_sb[32 * h:32 * h + 32, chunk * 512:(chunk + 1) * 512],
                    start=True, stop=True, tile_position=(32 * h, 0))
            ets = []
            for h in range(NHC):
                t = et_pool.tile([128, 512], F32, tag="et", name="et")
                ets.append(t)
                nc.scalar.activation(t[:], pss[h][:], EXP, scale=SCALE)
            for h in range(NHC):
                nc.tensor.matmul(
                    xg_ps[32 * h:32 * h + 32, :],
                    v_sb[mt][:, 32 * h:32 * h + 32], ets[h][:],
                    start=(mt == 0), stop=(mt == NT - 1),
                    tile_position=(0, 32 * h))
            if mt // 4 == chunk:
                j = mt - 4 * chunk
                for h in range(NHC):
                    el = eloc_pool.tile([128, 128], F32, tag="eloc", name="eloc")
                    nc.vector.tensor_tensor(
                        el[:], ets[h][:, j * 128:(j + 1) * 128], m16_sb[:], MUL)
                    nc.tensor.matmul(
                        xl_ps[32 * h:32 * h + 32, j * 128:(j + 1) * 128],
                        v_sb[mt][:, 32 * h:32 * h + 32], el[:],
                        start=True, stop=True, tile_position=(0, 32 * h))
                    nc.tensor.matmul(
                        sl_ps[32 * h:32 * h + 32, j * 128:(j + 1) * 128],
                        ones_sb[:, :], el[:],
                        start=True, stop=True, tile_position=(0, 32 * h))

        xc_slice = xc_sb[:, chunk * 512:(chunk + 1) * 512]
        nc.vector.tensor_mul(xc_slice, xg_ps[:], rg_sb[:])
        rl_sb = stg_pool.tile([128, 512], F32, tag="rl", name="rl_sb")
        nc.vector.reciprocal(rl_sb[:], sl_ps[:])
        tmp = stg_pool.tile([128, 512], F32, tag="tmp", name="tmp")
        nc.vector.tensor_mul(tmp[:], xl_ps[:], rl_sb[:])
        nc.vector.tensor_add(xc_slice, xc_slice, tmp[:])

    # ---------------- output projection (partial) ----------------
    if not FLAGS["passB"]:
        nc.gpsimd.memset(xc_sb[:], 0.0)
    for nt in range(NT) if FLAGS["proj"] else []:
        pp = ps_s.tile([128, C], F32, tag="ps_s", name="ps_s")
        nc.tensor.matmul(
            pp[:], xc_sb[:, nt * 128:(nt + 1) * 128], wp_sb[:],
            start=True, stop=True)
        ot = stg_pool.tile([128, C], F32, tag="ot", name="ot")
        nc.vector.tensor_copy(ot[:], pp[:])
        nc.sync.dma_start(pout[nt * 128:(nt + 1) * 128, :], ot[:])


_NC_CACHE = None


def _get_nc():
    global _NC_CACHE
    if _NC_CACHE is None:
        _NC_CACHE = _build()
    return _NC_CACHE


def _make_in_maps(inputs):
    x = np.ascontiguousarray(np.asarray(inputs["x"], dtype=np.float32))
    W_qkv = np.asarray(inputs["W_qkv"], dtype=np.float32)
    W_proj = np.asarray(inputs["W_proj"], dtype=np.float32)

    Wq, Wk, Wv = W_qkv[:, :C], W_qkv[:, C:2 * C], W_qkv[:, 2 * C:]
    m16 = np.zeros((128, 128), dtype=np.float32)
    for w in range(128 // WIN):
        m16[w * WIN:(w + 1) * WIN, w * WIN:(w + 1) * WIN] = 1.0
    ones32 = np.ones((128, 32), dtype=np.float32)
    # Rg broadcast selector: rg[32h+d, 128j+p] = rT[4j+h, p]
    msel = np.zeros((64, 128), dtype=np.float32)
    for j in range(4):
        for hh in range(4):
            msel[16 * j + 4 * j + hh, 32 * hh:32 * (hh + 1)] = 1.0

    in_maps = []
    for c in range(8):
        b, hg = c // 2, c % 2
        cols = slice(hg * 128, (hg + 1) * 128)
        in_maps.append({
            "xT": np.ascontiguousarray(x[b].T),
            "wq": np.ascontiguousarray(Wq[:, cols]),
            "wk": np.ascontiguousarray(Wk[:, cols]),
            "wv": np.ascontiguousarray(Wv[:, cols]),
            "wp": np.ascontiguousarray(W_proj[cols, :]),
            "m16": m16,
            "ones32": ones32,
            "msel": msel,
        })
    return in_maps


def kernel(x, W_qkv, W_proj, b_proj):
    b_proj = np.asarray(b_proj, dtype=np.float32)
    in_maps = _make_in_maps({"x": x, "W_qkv": W_qkv, "W_proj": W_proj})

    nc = _get_nc()
    res = run_bass_kernel_spmd(nc, in_maps, core_ids=list(range(8)))
    if res.exec_time_ns is not None:
        print(f"HW exec time: {res.exec_time_ns} ns")

    weights = np.empty((B, H, N, N), dtype=np.float32)
    x_out = np.empty((B, N, C), dtype=np.float32)
    for c in range(8):
        b, hg = c // 2, c % 2
        weights[b, hg * NHC:(hg + 1) * NHC] = res.results[c]["attn"]
    for b in range(B):
        x_out[b] = res.results[2 * b]["pout"] + res.results[2 * b + 1]["pout"] + b_proj
    return (x_out, weights)
